# revision 8
# baseline (speedup 1.0000x reference)
"""CRZ diagonal-unitary kernel for Trainium2 (8 NeuronCores) - v2 (fp16 planar).

The reference computes U @ x with U = diag(d); d depends only on the top two
bits of the row index (D=4096, DIM=2, WIRES=12, control 0, target 1, J=1):
  rows [0, 2048)    : d = 1
  rows [2048, 3072) : d = exp(-i*angle/2)
  rows [3072, 4096) : d = exp(+i*angle/2)
So the op is a per-row-group complex scalar multiply - pure elementwise work.

Empirical cost model of this axon/trn2 execution backend (measured by
in-NEFF repetition slope; see mb.py/exp2.py history):
  * every instruction costs ~28-35us fixed (tiny DVE op == 128-partition op
    == 1-descriptor DMA == 128-descriptor DMA), plus data time
    (~5.6us/MB DMA, ~4-9us per 2M-element DVE op);
  * a blocked semaphore wait costs ~12-16us and, crucially, serializes the
    whole dependency web: a connected producer/consumer graph executes at
    the SUM of its instruction costs (pipelining/double-buffering across
    engines gains nothing - measured repeatedly);
  * only fully INDEPENDENT instruction webs (no shared semaphores/buffers)
    overlap (e.g. an unsynchronized DMA stream hides completely under an
    unsynchronized DVE stream).

The "independent webs overlap" effect does NOT survive contact with real
synchronized kernels: a twin-web variant (v27: SP+DVE web next to ACT+Pool
web, fully disjoint) measured 2-3x WORSE than one serial web - concurrent
webs contend. So the shipped kernel (VARIANT v25_half) is the minimum
serial chain: per core ONE web - SP queue load -> DVE (tt prefill with a
stride-0 (b,d) broadcast + aliasing stt accumulate via a negative-stride
half-swapped view) -> SP store - in fp16 (harness gate is 2e-2 rel err;
fp16 end-to-end gives ~2e-4), planar (contiguous) SBUF layouts, and
combined-semaphore counting (loads and stores inc ONE semaphore; SP-queue
FIFO makes the thresholds unambiguous) for exactly 2 waits + 3 incs per
invocation. The device handles only the 2048 non-identity rows (256
rows/core); rows [0,2048) multiply by exactly 1.0, so the host emits them
directly during unshard. Host packs [xi|xr] fp16 partition-major and
interleaves the complex output during unshard.
Measured: 173-189us/invocation (slope, r_hi=408) vs 274550ns baseline;
serial-sum accounting: 4 instr x ~33us + 2 waits x ~14us + ~20us data.
build_v27 is kept for reference/A-B only.

Math per element: out_r = a*xr + b*xi ; out_i = a*xi + d*xr with per-core
(a,b,d) = (1,0,0) / (cos, +/-sin, -/+sin).
"""

import math
from contextlib import ExitStack

import numpy as np

import concourse.bass as bass
import concourse.mybir as mybir
from concourse.bass_utils import run_bass_kernel_spmd

P = 128
BATCH = 2048
D = 4096
N_CORES = 8
ROWS = D // N_CORES      # 512 rows/core (full variant)
NT = ROWS // P           # 4 tiles of 128 rows
f16 = mybir.dt.float16
f32 = mybir.dt.float32
mult = mybir.AluOpType.mult
add = mybir.AluOpType.add

# Chosen by interleaved on-device compare (see docstring): the twin-web v27
# measured 2-3x WORSE than the single-web v25 (concurrent webs contend in this
# backend), and half-data beats full-data. v25_half: the device applies the
# two non-trivial phase groups (rows 2048..4095, 256 rows/core); the identity
# rows (phase exactly 1) are assembled on the host during unshard.
VARIANT = "v25_half"

_CACHE = {}


def _views(xt, ot, Hh):
    x3 = xt[:, :].rearrange("p (j k) -> p j k", j=2)
    sw = bass.AP(
        tensor=x3.tensor,
        offset=x3.offset + Hh,
        ap=[list(x3.ap[0]), [-Hh, 2], list(x3.ap[2])],
    )
    o3 = ot[:, :].rearrange("p (j k) -> p j k", j=2)
    return x3, sw, o3


def _bd_of(ct, Hh):
    return ct[:, 1:3].rearrange("p (j o) -> p j o", j=2).broadcast_to((P, 2, Hh))


def build_v25(reps, bench, nt):
    """Single web: SP queue (load+store) + DVE (tt + stt). nt tiles/core."""
    key = ("v25", reps, bench, nt)
    if key in _CACHE:
        return _CACHE[key]
    H = nt * BATCH
    W2 = 2 * H

    nc = bass.Bass()
    big = "Internal" if bench else None
    xin = nc.dram_tensor("xin", [P, W2], f16, kind=big or "ExternalInput")
    coef = nc.dram_tensor("coef", [P, 4], f16, kind="ExternalInput")
    out = nc.dram_tensor("out", [P, W2], f16, kind=big or "ExternalOutput")
    if bench:
        out_small = nc.dram_tensor("out_small", [P, 4], f16, kind="ExternalOutput")

    with ExitStack() as stack:
        xin_t = stack.enter_context(nc.sbuf_tensor("xint", [P, W2], f16))
        out_t = stack.enter_context(nc.sbuf_tensor("outt", [P, W2], f16))
        coef_t = stack.enter_context(nc.sbuf_tensor("coeft", [P, 4], f16))
        ld_sem = stack.enter_context(nc.semaphore())
        dve_sem = stack.enter_context(nc.semaphore())
        st_sem = stack.enter_context(nc.semaphore())
        block = stack.enter_context(nc.Block())

        xin3, swap, out3p = _views(xin_t, out_t, H)
        a_ap = coef_t[:, 0:1]
        bd_pat = _bd_of(coef_t, H)

        @block.sync
        def _(sync):
            sync.dma_start(coef_t[:, :], coef[:, :]).then_inc(ld_sem, 16)
            for r in range(reps):
                sync.dma_start(xin_t[:, :], xin[:, :]).then_inc(ld_sem, 16)
                sync.wait_ge(dve_sem, r + 1)
                sync.dma_start(out[:, :], out_t[:, :]).then_inc(ld_sem, 16)
            if bench:
                sync.dma_start(out_small[:, :], coef_t[:, :]).then_inc(st_sem, 16)

        @block.vector
        def _(vector):
            for r in range(reps):
                vector.wait_ge(ld_sem, 16 * (2 * r + 2))
                nc.vector.tensor_tensor(out3p, xin3, bd_pat, op=mult)
                nc.vector.scalar_tensor_tensor(
                    out3p, swap, a_ap, out3p, op0=mult, op1=add
                ).then_inc(dve_sem, 1)

    _CACHE[key] = nc
    return nc


def build_v27(reps, bench, nt):
    """Twin independent webs (see module docstring). nt tiles/core total."""
    key = ("v27", reps, bench, nt)
    if key in _CACHE:
        return _CACHE[key]
    Hh = (nt // 2) * BATCH
    W2h = 2 * Hh

    nc = bass.Bass()
    big = "Internal" if bench else None
    xina = nc.dram_tensor("xina", [P, W2h], f16, kind=big or "ExternalInput")
    xinb = nc.dram_tensor("xinb", [P, W2h], f16, kind=big or "ExternalInput")
    coef = nc.dram_tensor("coef", [P, 4], f16, kind="ExternalInput")
    coef32 = nc.dram_tensor("coef32", [P, 4], f32, kind="ExternalInput")
    outa = nc.dram_tensor("outa", [P, W2h], f16, kind=big or "ExternalOutput")
    outb = nc.dram_tensor("outb", [P, W2h], f16, kind=big or "ExternalOutput")
    if bench:
        out_small = nc.dram_tensor("out_small", [P, 4], f16, kind="ExternalOutput")

    with ExitStack() as stack:
        xa_t = stack.enter_context(nc.sbuf_tensor("xa", [P, W2h], f16))
        oa_t = stack.enter_context(nc.sbuf_tensor("oa", [P, W2h], f16))
        xb_t = stack.enter_context(nc.sbuf_tensor("xb", [P, W2h], f16))
        ob_t = stack.enter_context(nc.sbuf_tensor("ob", [P, W2h], f16))
        tb_t = stack.enter_context(nc.sbuf_tensor("tb", [P, W2h], f16))
        coef_a = stack.enter_context(nc.sbuf_tensor("coefa", [P, 4], f16))
        coef_b = stack.enter_context(nc.sbuf_tensor("coefb", [P, 4], f16))
        coef_b32 = stack.enter_context(nc.sbuf_tensor("coefb32", [P, 4], f32))
        lda_sem = stack.enter_context(nc.semaphore())
        dvea_sem = stack.enter_context(nc.semaphore())
        ldb_sem = stack.enter_context(nc.semaphore())
        poolb_sem = stack.enter_context(nc.semaphore())
        block = stack.enter_context(nc.Block())

        xa3, swa, oa3 = _views(xa_t, oa_t, Hh)
        xb3, swb, ob3 = _views(xb_t, ob_t, Hh)
        tb3 = tb_t[:, :].rearrange("p (j k) -> p j k", j=2)
        a_a, bd_a = coef_a[:, 0:1], _bd_of(coef_a, Hh)
        a_b, bd_b = coef_b32[:, 0:1], _bd_of(coef_b, Hh)

        @block.sync
        def _(sync):
            sync.dma_start(coef_a[:, :], coef[:, :]).then_inc(lda_sem, 16)
            for r in range(reps):
                sync.dma_start(xa_t[:, :], xina[:, :]).then_inc(lda_sem, 16)
                sync.wait_ge(dvea_sem, r + 1)
                sync.dma_start(outa[:, :], oa_t[:, :]).then_inc(lda_sem, 16)
            if bench:
                sync.dma_start(out_small[:, :], coef_a[:, :]).then_inc(lda_sem, 16)

        @block.vector
        def _(vector):
            for r in range(reps):
                vector.wait_ge(lda_sem, 16 * (2 * r + 2))
                nc.vector.tensor_tensor(oa3, xa3, bd_a, op=mult)
                nc.vector.scalar_tensor_tensor(
                    oa3, swa, a_a, oa3, op0=mult, op1=add
                ).then_inc(dvea_sem, 1)

        @block.scalar
        def _(scalar):
            scalar.dma_start(coef_b[:, :], coef[:, :]).then_inc(ldb_sem, 16)
            scalar.dma_start(coef_b32[:, :], coef32[:, :]).then_inc(ldb_sem, 16)
            for r in range(reps):
                scalar.dma_start(xb_t[:, :], xinb[:, :]).then_inc(ldb_sem, 16)
                scalar.wait_ge(poolb_sem, r + 1)
                scalar.dma_start(outb[:, :], ob_t[:, :]).then_inc(ldb_sem, 16)

        @block.gpsimd
        def _(g):
            for r in range(reps):
                g.wait_ge(ldb_sem, 16 * (2 * r + 3))
                nc.gpsimd.tensor_tensor(ob3, xb3, bd_b, op=mult)
                nc.gpsimd.tensor_scalar_mul(tb3, swb, a_b)
                nc.gpsimd.tensor_tensor(ob3, tb3, ob3, op=add).then_inc(poolb_sem, 1)

    _CACHE[key] = nc
    return nc


def build_v30(reps, bench, a_scale=0.9375):
    """DEAD END - kept as documentation: this walrus build rejects custom-DVE
    ucode ops at codegen ("ISA wrong length", CoreV2GenImpl.cpp visitInstISA),
    so ln_bwd_dx cannot compile here. Do not enable.

    3-instruction variant: ONE fused DVE op (LN_BWD_DX ucode) does the whole
    complex multiply. Partition-split layout: partitions 0..63 compute the real
    plane, 64..127 the imag plane of the same 256 rows (inputs duplicated, so
    the load doubles to 4MB/core - cheaper than a second ~33us instruction).
      out_p = (in0_p - in1_p * s0_p) * a      (internally f32)
      p<64 : in0=xr, in1=xi, s0=-b/a -> a*xr + b*xi
      p>=64: in0=xi, in1=xr, s0=-d/a -> a*xi + d*xr
    `a` rides as the op's literal scale (the NEFF is angle-specific; it is
    compiled on first call either way)."""
    key = ("v30", reps, bench, float(a_scale))
    if key in _CACHE:
        return _CACHE[key]
    HLN = 4 * BATCH  # 8192 elements per half per partition

    nc = bass.Bass()
    big = "Internal" if bench else None
    xin = nc.dram_tensor("xin", [P, 2 * HLN], f16, kind=big or "ExternalInput")
    s0c = nc.dram_tensor("s0c", [P, 4], f32, kind="ExternalInput")
    out = nc.dram_tensor("out", [P, HLN], f16, kind=big or "ExternalOutput")
    if bench:
        out_small = nc.dram_tensor("out_small", [P, 4], f32, kind="ExternalOutput")

    with ExitStack() as stack:
        xin_t = stack.enter_context(nc.sbuf_tensor("xint", [P, 2 * HLN], f16))
        out_t = stack.enter_context(nc.sbuf_tensor("outt", [P, HLN], f16))
        s0_t = stack.enter_context(nc.sbuf_tensor("s0t", [P, 4], f32))
        ld_sem = stack.enter_context(nc.semaphore())
        dve_sem = stack.enter_context(nc.semaphore())
        block = stack.enter_context(nc.Block())

        @block.sync
        def _(sync):
            sync.dma_start(s0_t[:, :], s0c[:, :]).then_inc(ld_sem, 16)
            for r in range(reps):
                sync.dma_start(xin_t[:, :], xin[:, :]).then_inc(ld_sem, 16)
                sync.wait_ge(dve_sem, r + 1)
                sync.dma_start(out[:, :], out_t[:, :]).then_inc(ld_sem, 16)
            if bench:
                sync.dma_start(out_small[:, :], s0_t[:, :]).then_inc(ld_sem, 16)

        @block.vector
        def _(vector):
            for r in range(reps):
                vector.wait_ge(ld_sem, 16 * (2 * r + 2))
                nc.vector.ln_bwd_dx(
                    out_t[:, :],
                    xin_t[:, 0:HLN],
                    xin_t[:, HLN : 2 * HLN],
                    s0_t[:, 0:1],
                    0.0,
                    scale=float(a_scale),
                ).then_inc(dve_sem, 1)

    _CACHE[key] = nc
    return nc


def _build(reps=1, bench=False, variant=None):
    variant = variant or VARIANT
    if variant == "v30":
        return build_v30(reps, bench)
    fam, size = variant.split("_")
    nt = NT if size == "full" else NT // 2
    if fam == "v25":
        return build_v25(reps, bench, nt)
    return build_v27(reps, bench, nt)


def bench_in_maps():
    if VARIANT == "v30":
        s0c = np.zeros((P, 4), np.float32)
        return [{"s0c": s0c} for _ in range(N_CORES)]
    coef = np.zeros((P, 4), np.float16)
    coef[:, 0] = 1.0
    return [
        {"coef": coef, "coef32": coef.astype(np.float32)} for _ in range(N_CORES)
    ]


def _coef_for_rows(r0, c, s):
    if r0 < 2048:
        return (1.0, 0.0, 0.0)
    if r0 < 3072:
        return (c, s, -s)
    return (c, -s, s)


def _pack(xr_rows, xi_rows, nt):
    Hh = nt * BATCH
    xi_pm = xi_rows.reshape(nt, P, BATCH).transpose(1, 0, 2).reshape(P, Hh)
    xr_pm = xr_rows.reshape(nt, P, BATCH).transpose(1, 0, 2).reshape(P, Hh)
    return np.ascontiguousarray(
        np.concatenate([xi_pm, xr_pm], axis=1).astype(np.float16)
    )


def _unpack_into(out, o, r0, nt):
    rows = nt * P
    planes = (
        o.astype(np.float32).reshape(P, 2, nt, BATCH).transpose(1, 2, 0, 3)
    )
    out[r0 : r0 + rows, 0::2] = planes[0].reshape(rows, BATCH)
    out[r0 : r0 + rows, 1::2] = planes[1].reshape(rows, BATCH)


def _kernel_v30(xr, xi, angle, a, c, s):
    HLN = 4 * BATCH
    nc = build_v30(1, False, a_scale=a)
    in_maps = []
    for i in range(N_CORES):
        r0 = D // 2 + i * 256
        _, b_, d_ = _coef_for_rows(r0, c, s)
        xr_blk = xr[r0 : r0 + 256].reshape(64, HLN)
        xi_blk = xi[r0 : r0 + 256].reshape(64, HLN)
        xin = np.empty((P, 2 * HLN), np.float16)
        xin[0:64, 0:HLN] = xr_blk
        xin[0:64, HLN:] = xi_blk
        xin[64:128, 0:HLN] = xi_blk
        xin[64:128, HLN:] = xr_blk
        s0c = np.zeros((P, 4), np.float32)
        s0c[0:64, 0] = -b_ / a
        s0c[64:128, 0] = -d_ / a
        in_maps.append({"xin": xin, "s0c": s0c})

    def outs_ok(res):
        for i in range(N_CORES):
            o = res.results[i]["out"]
            if not np.isfinite(o).all():
                return False
            m = in_maps[i]
            dev = o[:, :64].astype(np.float32)
            exp = (
                m["xin"][:, :64].astype(np.float32)
                - m["xin"][:, HLN : HLN + 64].astype(np.float32)
                * m["s0c"][:, 0:1]
            ) * a
            if not np.allclose(dev, exp, atol=0.05, rtol=0.05):
                return False
        return True

    for _attempt in range(4):
        res = run_bass_kernel_spmd(nc, in_maps, core_ids=list(range(N_CORES)))
        if outs_ok(res):
            break

    out = np.empty((D, 2 * BATCH), np.float32)
    out[: D // 2, 0::2] = xr[: D // 2]
    out[: D // 2, 1::2] = xi[: D // 2]
    for i in range(N_CORES):
        r0 = D // 2 + i * 256
        o = res.results[i]["out"].astype(np.float32)
        out[r0 : r0 + 256, 0::2] = o[0:64].reshape(256, BATCH)
        out[r0 : r0 + 256, 1::2] = o[64:128].reshape(256, BATCH)
    return out.view(np.complex64)


def kernel(x_real, x_imag, angle):
    variant = VARIANT
    if variant == "v30":
        ang = 0.5 * float(np.asarray(angle).reshape(-1)[0])
        c, s = math.cos(ang), math.sin(ang)
        xr = np.asarray(x_real, dtype=np.float32)
        xi = np.asarray(x_imag, dtype=np.float32)
        if abs(c) >= 1e-3:
            return _kernel_v30(xr, xi, angle, c, c, s)
        variant = "v25_half"  # s0=-b/a ill-conditioned near cos=0
    fam, size = variant.split("_")
    half = size == "half"
    nt = NT // 2 if half else NT          # device tiles per core
    ang = 0.5 * float(np.asarray(angle).reshape(-1)[0])
    c, s = math.cos(ang), math.sin(ang)

    xr = np.asarray(x_real, dtype=np.float32)
    xi = np.asarray(x_imag, dtype=np.float32)
    nc = _build(1, False, variant)

    rows_per_core = nt * P
    base = D // 2 if half else 0

    in_maps = []
    for i in range(N_CORES):
        r0 = base + i * rows_per_core
        a_, b_, d_ = _coef_for_rows(r0, c, s)
        coef = np.zeros((P, 4), np.float16)
        coef[:, 0] = a_
        coef[:, 1] = b_
        coef[:, 2] = d_
        sl = slice(r0, r0 + rows_per_core)
        if fam == "v25":
            in_maps.append(
                {"xin": _pack(xr[sl], xi[sl], nt), "coef": coef}
            )
        else:
            nth = nt // 2
            mid = r0 + nth * P
            in_maps.append(
                {
                    "xina": _pack(
                        xr[r0:mid], xi[r0:mid], nth
                    ),
                    "xinb": _pack(
                        xr[mid : r0 + rows_per_core],
                        xi[mid : r0 + rows_per_core],
                        nth,
                    ),
                    "coef": coef,
                    "coef32": coef.astype(np.float32),
                }
            )

    # The PJRT execute path very occasionally returns uninitialized output
    # buffers (observed ~1/15 runs: NaNs in otherwise-deterministic output).
    # Guard: outputs must be finite AND a host-recomputed spot sample must
    # match; otherwise re-run the (identical) executable.
    def _device_outs_ok(res):
        for i in range(N_CORES):
            m = in_maps[i]
            a_ = float(m["coef"][0, 0])
            b_ = float(m["coef"][0, 1])
            d_ = float(m["coef"][0, 2])
            names = ("out",) if fam == "v25" else ("outa", "outb")
            xins = ("xin",) if fam == "v25" else ("xina", "xinb")
            for oname, xname in zip(names, xins):
                o = res.results[i][oname]
                if not np.isfinite(o).all():
                    return False
                Hh = o.shape[1] // 2
                xi_s = m[xname][:4, :64].astype(np.float32)
                xr_s = m[xname][:4, Hh : Hh + 64].astype(np.float32)
                dev = o[:4].astype(np.float32)
                exp_r = a_ * xr_s + b_ * xi_s
                exp_i = a_ * xi_s + d_ * xr_s
                if not (
                    np.allclose(dev[:, :64], exp_r, atol=0.05, rtol=0.05)
                    and np.allclose(dev[:, Hh : Hh + 64], exp_i, atol=0.05, rtol=0.05)
                ):
                    return False
        return True

    for _attempt in range(4):
        res = run_bass_kernel_spmd(nc, in_maps, core_ids=list(range(N_CORES)))
        if _device_outs_ok(res):
            break

    out = np.empty((D, 2 * BATCH), np.float32)
    if half:
        out[: D // 2, 0::2] = xr[: D // 2]
        out[: D // 2, 1::2] = xi[: D // 2]
    for i in range(N_CORES):
        r0 = base + i * rows_per_core
        if fam == "v25":
            _unpack_into(out, res.results[i]["out"], r0, nt)
        else:
            nth = nt // 2
            _unpack_into(out, res.results[i]["outa"], r0, nth)
            _unpack_into(out, res.results[i]["outb"], r0 + nth * P, nth)
    return out.view(np.complex64)
